# revision 1
# baseline (speedup 1.0000x reference)
"""Trainium2 Bass kernel for the Luong-attention LSTM decoder (nn_Decoder).

8-core strategy:
- Host folds Wa into the recurrence: attn@Wk_a = h2@R' + ctx@C' with
  R' = Wa_top@Wk_a + lstm_r, C' = Wa_bot@Wk_a. The x-projection (+bias) is
  host-precomputed. attn2/logits defer to a batched epilogue.
- Recurrence replicated on all cores (weight-stream-bound); state kept
  transposed (h2T/ctxT) as matmul lhsT; gates pipelined in h-quarters.
- Attention batch-sharded (8 batches/core); one AllGather per step
  reassembles ctxT.
- Epilogue: attn2 = [h2,ctx]@Wa, logits = attn2@fc_w with vocab column-sharded
  4000/core; host concatenates vocab shards.
"""
import sys

sys.path.insert(0, "/opt/trn_rl_repo")

import numpy as np
import concourse.bass as bass
import concourse.tile as tile
from concourse import bacc, mybir
from concourse.bass_utils import run_bass_kernel_spmd

B, T_IN, T_DEC = 64, 64, 47
V, E, H = 32000, 512, 1024
NC = 8
BL = B // NC
VS = V // NC
KC = H // 128
NPAIR = BL // 2
GS = VS // 8
F32 = mybir.dt.float32
F16 = mybir.dt.float16
BF16 = mybir.dt.bfloat16

_CACHE = {}


def _build():
    nc = bacc.Bacc(None, target_bir_lowering=False)

    wz_d = nc.dram_tensor("wz", [16, 128, 4096], BF16, kind="ExternalInput")
    xp_d = nc.dram_tensor("xp", [T_DEC, B, 4096], BF16, kind="ExternalInput")
    h2t0_d = nc.dram_tensor("h2t0", [KC, 128, 64], BF16, kind="ExternalInput")
    c0_d = nc.dram_tensor("c0", [B, H], F32, kind="ExternalInput")
    wm_d = nc.dram_tensor("wm", [KC, 128, H], F32, kind="ExternalInput")
    memo_d = nc.dram_tensor("memo", [BL, T_IN, H], F16, kind="ExternalInput")
    memof_d = nc.dram_tensor("memof", [BL, T_IN, H], F32, kind="ExternalInput")
    idf_d = nc.dram_tensor("idf", [T_IN, T_IN], F32, kind="ExternalInput")
    isel_d = nc.dram_tensor("isel", [B, 72], F32, kind="ExternalInput")
    wa_d = nc.dram_tensor("wa", [16, 128, H], BF16, kind="ExternalInput")
    fcw_d = nc.dram_tensor("fcw", [KC, 128, VS], BF16, kind="ExternalInput")
    fcbr_d = nc.dram_tensor("fcbr", [128, VS], F32, kind="ExternalInput")
    idb_d = nc.dram_tensor("idb", [128, 128], BF16, kind="ExternalInput")
    out_d = nc.dram_tensor("out", [B, T_DEC, VS], F32, kind="ExternalOutput")

    h2h_d = nc.dram_tensor("h2hist", [T_DEC, 128, KC * B], BF16)
    ctxh_d = nc.dram_tensor("ctxhist", [T_DEC, 128, NC * KC * BL], BF16)
    agin = [nc.dram_tensor(f"agin{t}", [128, KC * BL], BF16) for t in range(T_DEC)]
    agout = [
        nc.dram_tensor(f"agout{t}", [NC * 128, KC * BL], BF16, addr_space="Shared")
        for t in range(T_DEC)
    ]
    rg = [list(range(NC))]

    with tile.TileContext(nc) as tc:
        with (
            tc.tile_pool(name="one", bufs=1) as one,
            tc.tile_pool(name="work", bufs=1) as work,
            tc.tile_pool(name="gat", bufs=2) as gat,
            tc.tile_pool(name="gz", bufs=1) as gz,
            tc.tile_pool(name="zps", bufs=2, space="PSUM") as zps,
            tc.tile_pool(name="tps", bufs=2, space="PSUM") as tps,
            tc.tile_pool(name="aps", bufs=2, space="PSUM") as aps,
        ):
            # ---------------- resident tiles ----------------
            # h2tx: [128, chunk, 72]: cols 0:64 full h2T, 64:72 own-batch h2T
            h2tx = one.tile([128, KC, 64], BF16, tag="h2tx")
            nc.gpsimd.dma_start(h2tx[:], h2t0_d.rearrange("c p b -> p c b")[:, :, 0:64])
            h2own = one.tile([128, KC, BL], F16, tag="h2own")
            nc.vector.memset(h2own[:], 0.0)
            isel = one.tile([B, 72], F32, tag="isel")
            nc.gpsimd.dma_start(isel[:], isel_d[:])
            ctxt = one.tile([128, KC, NC, BL], BF16, tag="ctxt")
            nc.vector.memset(ctxt[:], 0.0)
            cst = one.tile([B, H], F32, tag="cst")
            nc.gpsimd.dma_start(cst[:], c0_d[:])

            # keys for own batches: keysK[:, c2, b, t]
            keysK = one.tile([128, KC, BL, T_IN], F16)
            with (
                tc.tile_pool(name="boot", bufs=1) as boot,
                tc.tile_pool(name="bootw", bufs=2) as bootw,
                tc.tile_pool(name="bootm", bufs=1) as bootm,
            ):
                memT = boot.tile([128, KC, BL * T_IN], F32)
                idf = boot.tile([T_IN, T_IN], F32, tag="idf")
                nc.gpsimd.dma_start(idf[:], idf_d[:])
                for b in range(BL):
                    mrow = bootm.tile([T_IN, H], F32, tag="mrow")
                    nc.sync.dma_start(mrow[:], memof_d[b])
                    for c in range(KC):
                        mtp = tps.tile([128, T_IN], F32, tag="tp")
                        nc.tensor.transpose(
                            mtp[:, 0:T_IN], mrow[:, c * 128:(c + 1) * 128], idf[:]
                        )
                        nc.vector.tensor_copy(
                            memT[:, c, b * T_IN:(b + 1) * T_IN], mtp[:, 0:T_IN]
                        )
                for c2 in range(KC):
                    kp = aps.tile([128, BL * T_IN], F32, tag="a")
                    for k in range(KC):
                        wmc = bootw.tile([128, 128], F32, tag="wmc")
                        nc.gpsimd.dma_start(wmc[:], wm_d[k, :, c2 * 128:(c2 + 1) * 128])
                        nc.tensor.matmul(
                            kp[:], wmc[:], memT[:, k],
                            start=(k == 0), stop=(k == KC - 1),
                        )
                    nc.scalar.copy(keysK[:, c2], kp[:].rearrange("p (b t) -> p b t", b=BL))

            wz = one.tile([128, 16, 4096], BF16)
            for k in range(16):
                nc.sync.dma_start(wz[:, k], wz_d[k])

            # mem pair-packed for ctx: memPK[(parity*64+t), pair, chunk, h]
            memPK = one.tile([128, NPAIR, KC, 128], F16)
            for b in range(BL):
                half = (b % 2) * 64
                nc.sync.dma_start(
                    memPK[half:half + 64, b // 2].rearrange("t c h -> t (c h)"),
                    memo_d[b],
                )

            ones64 = one.tile([B, 1], F32, tag="ones64")
            nc.vector.memset(ones64[:], 1.0)
            onesr = one.tile([1, B], F32, tag="onesr")
            nc.vector.memset(onesr[:], 1.0)
            alignZ = one.tile([128, BL], F16, tag="alignZ")
            nc.vector.memset(alignZ[:], 0.0)

            # ================= decode loop =================
            for t in range(T_DEC):
                xpt = work.tile([B, 4096], BF16, tag="xp")
                nc.sync.dma_start(xpt[:], xp_d[t])
                h2b = work.tile([B, H], F32, tag="h2b")
                for qp in range(2):  # quarter-pairs; h2 chunks first so the
                    # AG-dependent ctx chunks overlap the collective
                    zq2 = [zps.tile([B, 1024], F32, tag="zq", name=f"zq{t}_{qp}_{j}") for j in range(2)]
                    for ks, ke in ((0, KC), (KC, 16)):
                        for j in range(2):
                            q = 2 * qp + j
                            for k in range(ks, ke):
                                if k < KC:
                                    lhs = h2tx[:, k]
                                else:
                                    lhs = ctxt[:, k - KC].rearrange("p r w -> p (r w)")
                                nc.tensor.matmul(
                                    zq2[j][:, 0:512], lhs,
                                    wz[:, k, q * 1024:q * 1024 + 512],
                                    start=(k == 0), stop=(k == 15),
                                )
                                nc.tensor.matmul(
                                    zq2[j][:, 512:1024], lhs,
                                    wz[:, k, q * 1024 + 512:(q + 1) * 1024],
                                    start=(k == 0), stop=(k == 15),
                                )
                    for j in range(2):
                        q = 2 * qp + j
                        zq = zq2[j]
                    # z2 = z + xproj ; gate order within zq: i,f,g,o
                        z2 = gz.tile([B, 4, 256], F32, tag="z2")
                        nc.vector.scalar_tensor_tensor(
                            z2[:], zq[:].rearrange("b (g n) -> b g n", g=4),
                            1.0, xpt[:, q * 1024:(q + 1) * 1024].rearrange("b (g n) -> b g n", g=4),
                            mybir.AluOpType.mult, mybir.AluOpType.add,
                        )
                        sif = gat.tile([B, 512], F32, tag="sif")
                        nc.scalar.activation(
                            sif[:].rearrange("b (a n) -> b a n", a=2), z2[:, 0:2],
                            mybir.ActivationFunctionType.Sigmoid)
                        so = gat.tile([B, 256], F32, tag="so")
                        nc.scalar.activation(so[:], z2[:, 3],
                                             mybir.ActivationFunctionType.Sigmoid)
                        tg = gat.tile([B, 256], F32, tag="tg")
                        nc.scalar.activation(tg[:], z2[:, 2],
                                             mybir.ActivationFunctionType.Tanh)
                        qs = slice(q * 256, (q + 1) * 256)
                        nc.vector.tensor_mul(sif[:, 256:512], sif[:, 256:512], cst[:, qs])
                        nc.vector.tensor_mul(sif[:, 0:256], sif[:, 0:256], tg[:])
                        nc.vector.tensor_add(cst[:, qs], sif[:, 256:512], sif[:, 0:256])
                        nc.scalar.activation(tg[:], cst[:, qs],
                                             mybir.ActivationFunctionType.Tanh)
                        nc.vector.tensor_mul(h2b[:, qs], so[:], tg[:])

                # transpose h2 (+ own-col gather): [64,128] @ [64,72]
                for c in range(KC):
                    tp = tps.tile([128, 72], F32, tag="tp")
                    nc.tensor.matmul(tp[:], h2b[:, c * 128:(c + 1) * 128], isel[:],
                                     start=True, stop=True)
                    nc.scalar.copy(h2tx[:, c, :], tp[:, 0:64])
                    nc.vector.tensor_copy(h2own[:, c, :], tp[:, 64:72])
                nc.sync.dma_start(
                    h2h_d[t].rearrange("p (c b) -> p c b", c=KC), h2tx[:]
                )

                # ---- score (own batches): scT8[t, j] ----
                scT8 = aps.tile([64, BL], F32, tag="a")
                for j in range(BL):
                    for c in range(KC):
                        nc.tensor.matmul(
                            scT8[:, j:j + 1], keysK[:, c, j, :],
                            h2own[:, c, j:j + 1],
                            start=(c == 0), stop=(c == KC - 1),
                        )
                e8 = gat.tile([64, BL], F32, tag="e8")
                nc.scalar.activation(e8[:], scT8[:], mybir.ActivationFunctionType.Exp)
                s18 = tps.tile([1, BL], F32, tag="tp")
                nc.tensor.matmul(s18[:], ones64[:], e8[:], start=True, stop=True)
                r18 = gat.tile([1, BL], F32, tag="r18")
                nc.vector.reciprocal(r18[:], s18[:])
                rb = tps.tile([64, BL], F32, tag="tp")
                nc.tensor.matmul(rb[:], onesr[:], r18[:], start=True, stop=True)
                a8 = gat.tile([64, BL], F16, tag="a8")
                nc.vector.tensor_mul(a8[:], e8[:], rb[:])
                # scatter: even own-batches -> upper half, odd -> lower half
                nc.vector.tensor_copy(
                    alignZ[0:64, :].rearrange("p (pr two) -> p pr two", two=2)[:, :, 0],
                    a8[:].rearrange("p (pr two) -> p pr two", two=2)[:, :, 0],
                )
                nc.vector.tensor_copy(
                    alignZ[64:128, :].rearrange("p (pr two) -> p pr two", two=2)[:, :, 1],
                    a8[:].rearrange("p (pr two) -> p pr two", two=2)[:, :, 1],
                )

                # ---- ctx (own batches, pair-packed block-diag) ----
                ctxPS = aps.tile([128, KC, BL], F32, tag="a")
                for pr in range(NPAIR):
                    for c in range(KC):
                        nc.tensor.matmul(
                            ctxPS[:, c, 2 * pr:2 * pr + 2],
                            memPK[:, pr, c, :],
                            alignZ[:, 2 * pr:2 * pr + 2],
                            start=True, stop=True,
                        )
                ctxo = gat.tile([128, KC, BL], BF16, tag="ctxo")
                nc.scalar.copy(ctxo[:], ctxPS[:])

                # ---- AllGather ctx ----
                nc.gpsimd.dma_start(agin[t][:], ctxo[:].rearrange("p c w -> p (c w)"))
                nc.gpsimd.collective_compute(
                    "AllGather", mybir.AluOpType.bypass,
                    replica_groups=rg,
                    ins=[agin[t][:]], outs=[agout[t][:]],
                )
                for r in range(NC):
                    nc.gpsimd.dma_start(
                        ctxt[:, :, r, :],
                        agout[t][r * 128:(r + 1) * 128, :].rearrange("p (c w) -> p c w", c=KC),
                    )
                nc.sync.dma_start(
                    ctxh_d[t], ctxt[:].rearrange("p c r w -> p (c r w)")
                )

        # ================= epilogue =================
        with (
            tc.tile_pool(name="eone", bufs=1) as eone,
            tc.tile_pool(name="ework", bufs=3) as ework,
            tc.tile_pool(name="eps", bufs=2, space="PSUM") as eps,
            tc.tile_pool(name="fps", bufs=2, space="PSUM") as fps,
        ):
            wa_sb = eone.tile([128, 16, H], BF16)
            nc.sync.dma_start(wa_sb[:], wa_d.rearrange("k p h -> p k h"))
            fcw_sb = eone.tile([128, KC, VS], BF16)
            nc.sync.dma_start(fcw_sb[:], fcw_d.rearrange("k p v -> p k v"))
            identb = eone.tile([128, 128], BF16)
            nc.gpsimd.dma_start(identb[:], idb_d[:])
            fcbR = eone.tile([128, VS], F32)
            nc.gpsimd.dma_start(fcbR[:], fcbr_d[:])

            for p in range((T_DEC + 1) // 2):
                t0 = 2 * p
                nsteps = 2 if t0 + 1 < T_DEC else 1
                M = 64 * nsteps
                h2p = ework.tile([128, KC, 2, B], BF16, tag="h2p")
                ctxp = ework.tile([128, KC, 2, B], BF16, tag="ctxp")
                for i in range(nsteps):
                    nc.sync.dma_start(
                        h2p[:, :, i, :],
                        h2h_d[t0 + i].rearrange("p (c b) -> p c b", c=KC),
                    )
                    nc.sync.dma_start(
                        ctxp[:, :, i, :],
                        ctxh_d[t0 + i].rearrange("p (c b) -> p c b", c=KC),
                    )
                a2 = eps.tile([128, H], F32, tag="a2")
                for k in range(16):
                    if k < KC:
                        lhs = h2p[:, k, 0:nsteps, :].rearrange("p s b -> p (s b)")
                    else:
                        lhs = ctxp[:, k - KC, 0:nsteps, :].rearrange("p s b -> p (s b)")
                    for n in range(2):
                        nc.tensor.matmul(
                            a2[0:M, n * 512:(n + 1) * 512],
                            lhs, wa_sb[:, k, n * 512:(n + 1) * 512],
                            start=(k == 0), stop=(k == 15),
                        )
                a2sb = ework.tile([128, H], BF16, tag="a2sb")
                nc.scalar.copy(a2sb[0:M, :], a2[0:M, :])
                a2t = ework.tile([128, KC, 128], BF16, tag="a2t")
                for c2 in range(KC):
                    tp = eps.tile([128, 128], BF16, tag="a2tp")
                    nc.tensor.transpose(
                        tp[:, 0:M], a2sb[0:M, c2 * 128:(c2 + 1) * 128], identb[0:M, 0:M]
                    )
                    nc.scalar.copy(a2t[:, c2, 0:M], tp[:, 0:M])
                for g in range(8):
                    lg = fps.tile([128, GS], F32, tag="lg")
                    for k in range(KC):
                        nc.tensor.matmul(
                            lg[0:M, :], a2t[:, k, 0:M],
                            fcw_sb[:, k, g * GS:(g + 1) * GS],
                            start=(k == 0), stop=(k == KC - 1),
                        )
                    lgs = ework.tile([128, GS], F32, tag="lgs")
                    nc.vector.scalar_tensor_tensor(
                        lgs[0:M, :], lg[0:M, :], 1.0, fcbR[0:M, g * GS:(g + 1) * GS],
                        mybir.AluOpType.mult, mybir.AluOpType.add,
                    )
                    for i in range(nsteps):
                        nc.sync.dma_start(
                            out_d[:, t0 + i, g * GS:(g + 1) * GS],
                            lgs[i * 64:(i + 1) * 64, :],
                        )

    nc.finalize()
    return nc


def _prep_inputs(inputs):
    bfnp = mybir.dt.np(BF16)
    f32 = lambda x: np.asarray(x, dtype=np.float32)
    tokens = np.asarray(inputs["tokens"])
    memory = f32(inputs["memory"])
    enc_h = f32(inputs["enc_h"])
    enc_c = f32(inputs["enc_c"])
    emb = f32(inputs["emb"])
    Wm = f32(inputs["Wm"])
    Wa = f32(inputs["Wa"])
    lstm_k = f32(inputs["lstm_k"])
    lstm_r = f32(inputs["lstm_r"])
    lstm_b = f32(inputs["lstm_b"])
    fc_w = f32(inputs["fc_w"])
    fc_b = f32(inputs["fc_b"])

    Wk_x = lstm_k[:E]
    Wk_a = lstm_k[E:]
    Rp = Wa[:H] @ Wk_a + lstm_r
    Cp = Wa[H:] @ Wk_a
    wzf = np.concatenate([Rp, Cp], 0)
    wzf = wzf.reshape(2048, 4, 4, 256).transpose(0, 2, 1, 3).reshape(2048, 4096)
    wz = np.ascontiguousarray(wzf).reshape(16, 128, 4096).astype(bfnp)
    xs = emb[tokens]                                   # [B, T_DEC, E]
    xpb = xs @ Wk_x + lstm_b
    # t=0 folding correction: attn_0 = 0 (not [enc_h,0]@Wa) and h_0 = enc_h;
    # absorb enc_h@lstm_r into xproj[0] and start the device h2 state at zero.
    xpb[:, 0] += enc_h @ lstm_r
    xpf = xpb.transpose(1, 0, 2)
    xpf = xpf.reshape(T_DEC, B, 4, 4, 256).transpose(0, 1, 3, 2, 4)
    xp = np.ascontiguousarray(xpf).reshape(T_DEC, B, 4096).astype(bfnp)
    wm = np.ascontiguousarray(Wm.reshape(KC, 128, H), np.float32)
    wa = Wa.reshape(16, 128, H).astype(bfnp)
    idb = np.eye(128, dtype=np.float32).astype(bfnp)

    common = dict(wz=wz, xp=xp, wm=wm, wa=wa, c0=enc_c.copy(), idb=idb,
                  idf=np.eye(T_IN, dtype=np.float32))
    h2t_full = np.zeros((KC, 128, B), np.float32)
    maps = []
    for r in range(NC):
        own = slice(r * BL, (r + 1) * BL)
        sel = np.zeros((B, BL), np.float32)
        sel[np.arange(r * BL, (r + 1) * BL), np.arange(BL)] = 1.0
        isel = np.concatenate([np.eye(B, dtype=np.float32), sel], axis=1)
        h2t0 = h2t_full
        maps.append(dict(
            common,
            h2t0=np.ascontiguousarray(h2t0).astype(bfnp),
            memo=memory[own].astype(np.float16),
            memof=np.ascontiguousarray(memory[own], np.float32),
            isel=np.ascontiguousarray(isel),
            fcw=np.ascontiguousarray(
                fc_w[:, r * VS:(r + 1) * VS]).reshape(KC, 128, VS).astype(bfnp),
            fcbr=np.ascontiguousarray(
                np.broadcast_to(fc_b[r * VS:(r + 1) * VS], (128, VS)), np.float32),
        ))
    return maps


def kernel(**inputs):
    if "nc" not in _CACHE:
        _CACHE["nc"] = _build()
    nc = _CACHE["nc"]
    maps = _prep_inputs(inputs)
    res = run_bass_kernel_spmd(nc, maps, list(range(NC)))
    global LAST_RESULT
    LAST_RESULT = res
    out = np.concatenate([res.results[r]["out"] for r in range(NC)], axis=2)
    return out


LAST_RESULT = None



# revision 18
# speedup vs baseline: 1.5227x; 1.5227x over previous
"""Trainium2 Bass kernel for the Luong-attention LSTM decoder (nn_Decoder).

8-core strategy (v2, gate-sharded recurrence):
- Host folds Wa into the recurrence (z = h2T@R' + ctxT@C', R' = Wa_top@Wk_a
  + lstm_r, C' = Wa_bot@Wk_a); x-projection precomputed on host.
- The 4096 gate dims are sharded 8-way: core r computes gates i,f,o,g for
  h-dims r*128:(r+1)*128 only (z-slice [64,512]) -> 8x less tensor work
  than replication. Per step the h2T slice [128,64] is AllGathered (AG1).
- Sigmoid is computed as 0.5*(1+tanh(x/2)) so the scalar engine needs only
  the exp/tanh activation table (no per-step table reloads). The device
  h-state carries 2*h2; host pre-scales R', keys and Wa_top by 0.5.
- Attention batch-sharded: scores for own 8 batches via a diag-trick
  (8 matmuls vs full gathered h2T), own-row extraction via per-core 0/1
  sel matrix (keeps the SPMD program position-independent), softmax,
  pair-packed ctx, then ctx AllGathered (AG2).
- Epilogue (attn2 = [h2,ctx]@Wa, logits = attn2@fc_w vocab-sharded
  4000/core) is interleaved into the decode loop in ~7us chunks that run
  while the AllGathers are in flight; logits are written in bf16.
"""
import sys

sys.path.insert(0, "/opt/trn_rl_repo")

import numpy as np
import concourse.bass as bass
import concourse.tile as tile
from concourse import bacc, mybir
from concourse.bass_utils import run_bass_kernel_spmd

B, T_IN, T_DEC = 64, 64, 47
V, E, H = 32000, 512, 1024
NC = 8
BL = B // NC          # own batches per core
VS = V // NC          # vocab slice per core
KC = H // 128         # h chunks
GS = VS // 8          # vocab group
F32 = mybir.dt.float32
F16 = mybir.dt.float16
BF16 = mybir.dt.bfloat16
AF = mybir.ActivationFunctionType
ALU = mybir.AluOpType

_CACHE = {}


def _build():
    nc = bacc.Bacc(None, target_bir_lowering=False)

    wz_d = nc.dram_tensor("wz", [16, 128, 512], BF16, kind="ExternalInput")
    xp_d = nc.dram_tensor("xp", [T_DEC, B, 512], BF16, kind="ExternalInput")
    c0_d = nc.dram_tensor("c0", [B, 128], F32, kind="ExternalInput")
    keys_d = nc.dram_tensor("keys", [128, KC, BL * T_IN], BF16, kind="ExternalInput")
    mpk_d = nc.dram_tensor("mpk", [128, BL // 2, KC, 128], F16, kind="ExternalInput")
    sel_d = nc.dram_tensor("sel", [B, BL], F32, kind="ExternalInput")
    idb_d = nc.dram_tensor("idb", [128, 128], BF16, kind="ExternalInput")
    wa_d = nc.dram_tensor("wa", [16, 128, H], BF16, kind="ExternalInput")
    fcw_d = nc.dram_tensor("fcw", [KC, 128, VS], BF16, kind="ExternalInput")
    fcb_d = nc.dram_tensor("fcb", [128, VS], BF16, kind="ExternalInput")
    out_d = nc.dram_tensor("out", [B, T_DEC, VS], BF16, kind="ExternalOutput")

    agin1 = [nc.dram_tensor(f"agh{t}", [128, B], BF16) for t in range(T_DEC)]
    agout1 = [
        nc.dram_tensor(f"agho{t}", [NC * 128, B], BF16, addr_space="Shared")
        for t in range(T_DEC)
    ]
    agin2 = [nc.dram_tensor(f"agc{t}", [128, B], BF16) for t in range(T_DEC)]
    agout2 = [
        nc.dram_tensor(f"agco{t}", [NC * 128, B], BF16, addr_space="Shared")
        for t in range(T_DEC)
    ]
    rg = [list(range(NC))]

    with tile.TileContext(nc) as tc:
        with (
            tc.tile_pool(name="one", bufs=1) as one,
            tc.tile_pool(name="work", bufs=2) as work,
            tc.tile_pool(name="ga", bufs=2) as ga,
            tc.tile_pool(name="ep", bufs=2) as ep,
            tc.tile_pool(name="zps", bufs=1, space="PSUM") as zps,
            tc.tile_pool(name="scps", bufs=1, space="PSUM") as scps,
            tc.tile_pool(name="auxp", bufs=1, space="PSUM") as auxp,
            tc.tile_pool(name="a2ps", bufs=1, space="PSUM") as a2ps,
            tc.tile_pool(name="fcps", bufs=2, space="PSUM") as fcps,
        ):
            # ---------------- resident tiles ----------------
            cst = one.tile([B, 128], F32, tag="cst")
            nc.sync.dma_start(cst[:], c0_d[:])
            keysK = one.tile([128, KC, BL * T_IN], BF16, tag="keysK")
            nc.sync.dma_start(keysK[:], keys_d[:])
            memPK = one.tile([128, BL // 2, KC, 128], F16, tag="memPK")
            nc.sync.dma_start(memPK[:], mpk_d[:])
            sel8 = one.tile([B, BL], F32, tag="sel8")
            nc.gpsimd.dma_start(sel8[:], sel_d[:])
            idb = one.tile([128, 128], BF16, tag="idb")
            nc.gpsimd.dma_start(idb[:], idb_d[:])
            wz_sb = one.tile([128, 16, 512], BF16, tag="wz")
            nc.gpsimd.dma_start(wz_sb[:], wz_d.rearrange("k p n -> p k n"))
            wa_sb = one.tile([128, 16, H], BF16, tag="wa")
            nc.scalar.dma_start(wa_sb[:], wa_d.rearrange("k p n -> p k n"))
            fcw_sb = one.tile([128, KC, VS], BF16, tag="fcw")
            nc.scalar.dma_start(fcw_sb[:], fcw_d.rearrange("k p v -> p k v"))
            fcbR = one.tile([128, VS], BF16, tag="fcb")
            nc.scalar.dma_start(fcbR[:], fcb_d[:])

            # state: full h2T (x2 scale), full ctxT; layouts chosen so the
            # AllGather reassembly DMA has 128B-contiguous runs.
            h2tx = one.tile([128, KC, B], BF16, tag="h2tx")
            nc.vector.memset(h2tx[:], 0.0)
            # qctx[p, q, c, w]: raw gathered ctx payloads (q-major, DMA-friendly)
            qctx = one.tile([128, NC, KC, BL], BF16, tag="qctx")
            # ctxt[p, c, b]: reordered full ctxT used as matmul lhsT
            ctxt = one.tile([128, KC, B], BF16, tag="ctxt")
            nc.vector.memset(ctxt[:], 0.0)
            alignZ = one.tile([128, BL], F16, tag="alignZ")
            nc.vector.memset(alignZ[:], 0.0)
            ones64 = one.tile([B, 1], F32, tag="ones64")
            nc.vector.memset(ones64[:], 1.0)
            onesr = one.tile([1, B], F32, tag="onesr")
            nc.vector.memset(onesr[:], 1.0)

            # epilogue lag buffers (4 slots)
            h2pair = one.tile([128, KC, 4, B], BF16, tag="h2pair")
            ctxpair = one.tile([128, KC, 4, B], BF16, tag="ctxpair")

            # one shared PSUM scratch bank: transposes, softmax aux, ctx
            aux = auxp.tile([128, 512], F32, tag="aux")
            scT8ap = aux[0:B, 0:BL]
            s18ap = aux[0:1, 8:16]
            rbap = aux[0:B, 16:24]
            tpap = aux[:, 24:56].bitcast(BF16)            # [128, 64] bf16
            a2tpap = aux[:, 64:128].bitcast(BF16)         # [128, 128] bf16
            ctxPSap = aux[:, 128:192].rearrange("p (c w) -> p c w", c=KC)

            # ---------------- epilogue chunk emitters ----------------
            a2t_ref = {}

            def ep_a2(t0, s0, nsteps):
                def f():
                    M = 64 * nsteps
                    a2t = ep.tile([128, KC, 128], BF16, tag="a2t",
                                  name=f"a2t_{t0}")
                    a2t_ref[t0] = a2t
                    for half in range(2):
                        apt = a2ps.tile([128, 512], F32, tag="a2ps",
                                        name=f"a2ps_{t0}_{half}")
                        for k in range(16):
                            if k < KC:
                                lhs = h2pair[:, k, s0:s0 + nsteps, :].rearrange(
                                    "p s b -> p (s b)")
                            else:
                                lhs = ctxpair[:, k - KC, s0:s0 + nsteps, :
                                              ].rearrange("p s b -> p (s b)")
                            nc.tensor.matmul(
                                apt[0:M, :], lhs,
                                wa_sb[:, k, half * 512:(half + 1) * 512],
                                start=(k == 0), stop=(k == 15),
                            )
                        a2sb = ep.tile([128, 512], BF16, tag="a2sb",
                                       name=f"a2sb_{t0}_{half}")
                        nc.scalar.copy(a2sb[0:M, :], apt[0:M, :])
                        for c2 in range(4):
                            cc = half * 4 + c2
                            nc.tensor.transpose(
                                a2tpap[:, 0:M], a2sb[0:M, c2 * 128:(c2 + 1) * 128],
                                idb[0:M, 0:M])
                            nc.vector.tensor_copy(a2t[:, cc, 0:M], a2tpap[:, 0:M])
                return f

            def ep_fc(t0, s0, nsteps, ghalf):
                def f():
                    M = 64 * nsteps
                    a2t = a2t_ref[t0]
                    for g in range(ghalf * 4, ghalf * 4 + 4):
                        lg = fcps.tile([128, GS], F32, tag="lg",
                                       name=f"lg_{t0}_{g}")
                        for k in range(KC):
                            nc.tensor.matmul(
                                lg[0:M, :], a2t[:, k, 0:M],
                                fcw_sb[:, k, g * GS:(g + 1) * GS],
                                start=(k == 0), stop=(k == KC - 1),
                            )
                        lgs = ep.tile([128, GS], BF16, tag="lgs",
                                      name=f"lgs_{t0}_{g}")
                        nc.vector.scalar_tensor_tensor(
                            lgs[0:M, :], lg[0:M, :], 1.0,
                            fcbR[0:M, g * GS:(g + 1) * GS], ALU.mult, ALU.add)
                        nc.scalar.dma_start(
                            out_d[:, t0:t0 + nsteps, g * GS:(g + 1) * GS
                                  ].rearrange("b s v -> s b v"),
                            lgs[0:M, :])
                return f

            pending = []

            def window():
                if pending:
                    pending.pop(0)()

            # ================= decode loop =================
            for t in range(T_DEC):
                xpt = work.tile([B, 512], BF16, tag="xp", name=f"xp{t}")
                nc.sync.dma_start(xpt[:], xp_d[t])

                if t > 0:
                    zq = zps.tile([B, 512], F32, tag="zq", name=f"zq{t}")
                    for k in range(16):
                        if k < KC:
                            lhs = h2tx[:, k]
                        else:
                            lhs = ctxt[:, k - KC]
                        nc.tensor.matmul(zq[:], lhs, wz_sb[:, k],
                                         start=(k == 0), stop=(k == 15))
                    z2 = ga.tile([B, 512], F32, tag="z2", name=f"z2{t}")
                    nc.vector.scalar_tensor_tensor(
                        z2[:], zq[:], 1.0, xpt[:], ALU.mult, ALU.add)
                    zin = z2
                else:
                    zin = xpt

                # gates: i,f,o via 0.5*(1+tanh(x/2)); col order i,f,o,g
                tfio = ga.tile([B, 384], F32, tag="tfio", name=f"tfio{t}")
                nc.scalar.activation(tfio[:], zin[:, 0:384], AF.Tanh, scale=0.5)
                tg = ga.tile([B, 128], F32, tag="tg", name=f"tg{t}")
                nc.scalar.activation(tg[:], zin[:, 384:512], AF.Tanh)
                sf = ga.tile([B, 128], F32, tag="sf", name=f"sf{t}")
                nc.vector.scalar_tensor_tensor(
                    sf[:], tfio[:, 128:256], 1.0, cst[:], ALU.add, ALU.mult)
                si = ga.tile([B, 128], F32, tag="si", name=f"si{t}")
                nc.vector.scalar_tensor_tensor(
                    si[:], tfio[:, 0:128], 1.0, tg[:], ALU.add, ALU.mult)
                nc.vector.scalar_tensor_tensor(
                    cst[:], sf[:], 0.5, si[:], ALU.mult, ALU.add)
                tc2 = ga.tile([B, 128], F32, tag="tc2", name=f"tc2{t}")
                nc.scalar.activation(tc2[:], cst[:], AF.Tanh, scale=0.5)
                h1b = ga.tile([B, 128], BF16, tag="h1b", name=f"h1b{t}")
                nc.vector.scalar_tensor_tensor(
                    h1b[:], tfio[:, 256:384], 1.0, tc2[:], ALU.add, ALU.mult)

                # transpose local h2T slice, AllGather it
                nc.tensor.transpose(tpap[:], h1b[:], idb[0:64, 0:64])
                stg1 = ga.tile([128, B], BF16, tag="stg1", name=f"stg1{t}")
                nc.scalar.copy(stg1[:], tpap[:])
                nc.gpsimd.dma_start(agin1[t][:], stg1[:])
                nc.gpsimd.collective_compute(
                    "AllGather", ALU.bypass, replica_groups=rg,
                    ins=[agin1[t][:]], outs=[agout1[t][:]])
                nc.sync.dma_start(
                    h2tx[:, 0:4, :],
                    agout1[t][0:512, :].rearrange("(c p) b -> p c b", p=128))
                nc.gpsimd.dma_start(
                    h2tx[:, 4:8, :],
                    agout1[t][512:1024, :].rearrange("(c p) b -> p c b", p=128))

                window()  # epilogue chunk runs during AG1 flight

                nc.vector.tensor_copy(h2pair[:, :, t % 4, :], h2tx[:])

                # scores (own batches): diag trick then sel extraction
                scG = scps.tile([B, 512], F32, tag="scG", name=f"scG{t}")
                for c in range(KC):
                    nc.tensor.matmul(scG[:], h2tx[:, c], keysK[:, c],
                                     start=(c == 0), stop=(c == KC - 1))
                scGs = ga.tile([B, 512], F32, tag="scGs", name=f"scGs{t}")
                nc.scalar.copy(scGs[:], scG[:])
                for j in range(BL):
                    nc.tensor.matmul(
                        scT8ap[:, j:j + 1], scGs[:, j * 64:(j + 1) * 64],
                        sel8[:, j:j + 1], start=True, stop=True)
                e8 = ga.tile([B, BL], F32, tag="e8", name=f"e8{t}")
                nc.scalar.activation(e8[:], scT8ap, AF.Exp)
                nc.tensor.matmul(s18ap, ones64[:], e8[:], start=True, stop=True)
                r18 = ga.tile([1, BL], F32, tag="r18", name=f"r18{t}")
                nc.vector.reciprocal(r18[:], s18ap)
                nc.tensor.matmul(rbap, onesr[:], r18[:], start=True, stop=True)
                a8 = ga.tile([B, BL], F16, tag="a8", name=f"a8{t}")
                nc.vector.tensor_mul(a8[:], e8[:], rbap)
                # scatter: even own-batches -> rows 0:64, odd -> rows 64:128
                nc.vector.tensor_copy(
                    alignZ[0:64, :].rearrange("p (pr two) -> p pr two", two=2)[:, :, 0],
                    a8[:].rearrange("p (pr two) -> p pr two", two=2)[:, :, 0])
                nc.vector.tensor_copy(
                    alignZ[64:128, :].rearrange("p (pr two) -> p pr two", two=2)[:, :, 1],
                    a8[:].rearrange("p (pr two) -> p pr two", two=2)[:, :, 1])

                # ctx (own batches, pair-packed block-diag)
                for pr in range(BL // 2):
                    for c in range(KC):
                        nc.tensor.matmul(
                            ctxPSap[:, c, 2 * pr:2 * pr + 2],
                            memPK[:, pr, c, :], alignZ[:, 2 * pr:2 * pr + 2],
                            start=True, stop=True)
                ctxo = ga.tile([128, KC * BL], BF16, tag="ctxo", name=f"ctxo{t}")
                nc.scalar.copy(ctxo[:], aux[:, 128:192])
                nc.gpsimd.dma_start(agin2[t][:], ctxo[:])
                nc.gpsimd.collective_compute(
                    "AllGather", ALU.bypass, replica_groups=rg,
                    ins=[agin2[t][:]], outs=[agout2[t][:]])
                nc.sync.dma_start(
                    qctx[:, 0:4, :, :],
                    agout2[t][0:512, :].rearrange("(q p) (c w) -> p q c w",
                                                  p=128, c=KC))
                nc.gpsimd.dma_start(
                    qctx[:, 4:8, :, :],
                    agout2[t][512:1024, :].rearrange("(q p) (c w) -> p q c w",
                                                     p=128, c=KC))
                nc.vector.tensor_copy(
                    ctxt[:].rearrange("p c (q w) -> p c q w", q=NC),
                    qctx[:].rearrange("p q c w -> p c q w"))

                window()  # epilogue chunk runs during AG2 flight

                nc.scalar.copy(ctxpair[:, :, t % 4, :], ctxt[:])

                if t % 2 == 1:
                    t0, s0 = t - 1, (t - 1) % 4
                    pending.append(ep_a2(t0, s0, 2))
                    pending.append(ep_fc(t0, s0, 2, 0))
                    pending.append(ep_fc(t0, s0, 2, 1))

            # ================= tail =================
            while pending:
                pending.pop(0)()
            t0, s0 = T_DEC - 1, (T_DEC - 1) % 4
            ep_a2(t0, s0, 1)()
            ep_fc(t0, s0, 1, 0)()
            ep_fc(t0, s0, 1, 1)()

    nc.finalize()
    return nc


def _prep_inputs(inputs):
    bfnp = mybir.dt.np(BF16)
    f32 = lambda x: np.asarray(x, dtype=np.float32)
    tokens = np.asarray(inputs["tokens"])
    memory = f32(inputs["memory"])
    enc_h = f32(inputs["enc_h"])
    enc_c = f32(inputs["enc_c"])
    emb = f32(inputs["emb"])
    Wm = f32(inputs["Wm"])
    Wa = f32(inputs["Wa"])
    lstm_k = f32(inputs["lstm_k"])
    lstm_r = f32(inputs["lstm_r"])
    lstm_b = f32(inputs["lstm_b"])
    fc_w = f32(inputs["fc_w"])
    fc_b = f32(inputs["fc_b"])

    Wk_x = lstm_k[:E]
    Wk_a = lstm_k[E:]
    # device h-state carries 2*h2 -> scale its consumers by 0.5
    Rp = 0.5 * (Wa[:H] @ Wk_a + lstm_r)
    Cp = Wa[H:] @ Wk_a
    wzf = np.concatenate([Rp, Cp], 0)                 # [2048, 4096]

    xpb = emb[tokens] @ Wk_x + lstm_b                 # [B, T_DEC, 4096]
    # t=0 folding: attn_0 = 0, h_0 = enc_h absorbed into xproj[0]
    xpb[:, 0] += enc_h @ lstm_r
    xpf = np.ascontiguousarray(xpb.transpose(1, 0, 2))  # [T_DEC, B, 4096]

    keys = 0.5 * (memory.reshape(-1, H) @ Wm).reshape(B, T_IN, H)

    waf = Wa.copy()
    waf[:H] *= 0.5
    wa = np.ascontiguousarray(waf.reshape(16, 128, H)).astype(bfnp)
    idb = np.eye(128, dtype=np.float32).astype(bfnp)

    common = dict(wa=wa, idb=idb)
    maps = []
    for r in range(NC):
        hs = slice(r * 128, (r + 1) * 128)
        cols = np.concatenate(
            [np.arange(g * H + r * 128, g * H + (r + 1) * 128) for g in (0, 1, 3, 2)])
        own = slice(r * BL, (r + 1) * BL)

        wz = np.ascontiguousarray(wzf[:, cols]).reshape(16, 128, 512).astype(bfnp)
        xp = np.ascontiguousarray(xpf[:, :, cols]).astype(bfnp)
        c0 = np.ascontiguousarray(2.0 * enc_c[:, hs], np.float32)

        ko = keys[own]                                 # [8, 64, 1024]
        kk = ko.transpose(2, 0, 1).reshape(KC, 128, BL, T_IN)
        kk = np.ascontiguousarray(kk.transpose(1, 0, 2, 3)).reshape(128, KC, BL * T_IN)
        mo = memory[own]                               # [8, 64, 1024]
        mp = mo.reshape(BL // 2, 2, T_IN, KC, 128).transpose(1, 2, 0, 3, 4)
        mp = np.ascontiguousarray(mp).reshape(128, BL // 2, KC, 128)

        sel = np.zeros((B, BL), np.float32)
        sel[np.arange(r * BL, (r + 1) * BL), np.arange(BL)] = 1.0

        maps.append(dict(
            common,
            wz=wz, xp=xp, c0=c0,
            keys=kk.astype(bfnp),
            mpk=mp.astype(np.float16),
            sel=sel,
            fcw=np.ascontiguousarray(
                fc_w[:, r * VS:(r + 1) * VS]).reshape(KC, 128, VS).astype(bfnp),
            fcb=np.ascontiguousarray(
                np.broadcast_to(fc_b[r * VS:(r + 1) * VS], (128, VS))).astype(bfnp),
        ))
    return maps


def kernel(**inputs):
    if "nc" not in _CACHE:
        _CACHE["nc"] = _build()
    nc = _CACHE["nc"]
    maps = _prep_inputs(inputs)
    res = run_bass_kernel_spmd(nc, maps, list(range(NC)))
    global LAST_RESULT
    LAST_RESULT = res
    out = np.concatenate([res.results[r]["out"] for r in range(NC)], axis=2)
    return np.asarray(out, dtype=np.float32)


LAST_RESULT = None
